# revision 100
# baseline (speedup 1.0000x reference)
"""Multi-head attention (B=2, SQ=SK=2048, D=1024, H=16, DK=64) on 8 TRN2 cores.

Sharding: core c handles batch b = c//4 and head-group hg = c%4 (4 heads,
256 feature columns of each projection).  Each core computes its heads'
Q/K/V projections, causal+padding-masked softmax attention, and a partial
output projection; the host sums the 4 partials per batch.

Schedule: the kernel is emitted q-chunk-major with K/Q projections,
V projection k-tile batches, attention, and the (one chunk deferred)
output projection interleaved:
    K0 Q0 V0 A0 | K1 Q1 V1 A1 O0 | ... | K3 Q3 V3 A3 O2 | O3
so the scalar-engine exp chain (the attention pacer) starts as soon as
the first 512 tokens of K and Q are projected, and all PE work after
that point fills the gaps the exp chain leaves.  All x tiles stay
resident in SBUF (bf16) so no DMA ever waits on compute.

Precision/rate strategy (all inputs converted on the host):
  projections + scores            bf16 operands, f32 PSUM
  AV (attn @ V), chunks >= 1      fp8 e4m3 via DoubleRow perf mode
                                  (2 k-tiles per matmul at 0.5 cyc/row)
  AV chunk 0                      bf16 (short causal rows would expose
                                  fp8 quantization noise un-averaged)
  p = exp(s/8 - 4.75)             bias cancels in the softmax ratio and
                                  keeps p_max well under the HW e4m3
                                  convert's inf threshold (240, not the
                                  448 e4m3fn max); max s/8 here is 9.29
  ctx, Wo, O-proj                 f32/f32r (full PE rate at N>=256)
  out                             bf16 partials, host sums in f32

Device layouts (per core):
  qT     [128, head, tok]  dk=64 in one 64-partition half of a 128 slot,
                           other half zeroed so score matmuls contract a
                           full K=128 (K<128 matmuls don't register for
                           the PE clock gate and run at half clock)
  kT     [128, pair, tok]  head PAIRS stacked un-masked: the qT side's
                           zero half already excludes the foreign head
                           from the contraction, halving K evictions
  v      [ktok, kt, head, 128]  fp8, rows padded to 128 (ISA rejects
                           65-wide DoubleRow LdWeights); mask folded in;
                           col 64 = masked ones -> softmax denominator
  sT     [ktok, qtok]      transposed scores (PSUM f32)
  pT     exp(sT/8-4.75)    causal: affine_select on diagonal blocks; the
                           fully-masked q-quarter of the last block is
                           memset instead of exp'd (scalar is the pacer)
  ctxT   [dk+1, qtok]      PSUM f32 accumulated over k-tile (pairs)
  out    [qtok, D]         bf16 partial, host sums the 4 head-groups
"""

import numpy as np

B, SQ, SK, D, H, DK = 2, 2048, 2048, 1024, 16, 64
N_CORES = 8
CORES_PER_BATCH = 4
DKC = D // CORES_PER_BATCH          # 256 projection columns per core
QCH = 512                           # q-chunk (moving free dim)
DEN_EPS = 1e-9
EXP_BIAS = -4.75

_PROG_CACHE = {}


def _build(cfg):
    """Build the per-core Bass program. cfg = (sq, sk, d, dkc)."""
    import concourse.bass as bass  # noqa: F401
    import concourse.mybir as mybir
    import concourse.tile as tile
    from concourse import bacc
    from contextlib import ExitStack

    f32 = mybir.dt.float32
    f32r = mybir.dt.float32r
    bf16 = mybir.dt.bfloat16
    fp8 = mybir.dt.float8e4
    i32 = mybir.dt.int32
    Exp = mybir.ActivationFunctionType.Exp
    mult = mybir.AluOpType.mult
    is_ge = mybir.AluOpType.is_ge
    DR = mybir.MatmulPerfMode.DoubleRow

    sq, sk, d, dkc = cfg
    kc_n = d // 128                  # contraction chunks for projections
    mc_n = dkc // 128                # 128-wide dk chunks (q/k layout)
    kt_n = sk // 128                 # key tiles
    qc_n = sq // QCH                 # q chunks
    hpc = dkc // DK                  # heads per core
    vw = DK + 1                      # v row width per head incl. ones col
    fc_n = d // 512                  # output feature chunks
    ktb = QCH // 128                 # v k-tiles per interleave batch

    nc = bacc.Bacc("TRN2", target_bir_lowering=False, debug=False,
                   enable_asserts=False, num_devices=N_CORES)

    xqT = nc.dram_tensor("xqT", [d, sq], bf16, kind="ExternalInput").ap()
    xkT = nc.dram_tensor("xkT", [d, sk], bf16, kind="ExternalInput").ap()
    xvT = nc.dram_tensor("xvT", [d, sk], bf16, kind="ExternalInput").ap()
    wq_d = nc.dram_tensor("wq", [d, dkc], bf16, kind="ExternalInput").ap()
    wk_d = nc.dram_tensor("wk", [d, dkc], bf16, kind="ExternalInput").ap()
    wv_d = nc.dram_tensor("wv", [d, dkc], bf16, kind="ExternalInput").ap()
    wo_d = nc.dram_tensor("wo", [dkc, d], f32r, kind="ExternalInput").ap()
    mask_d = nc.dram_tensor("maskb", [sk], i32, kind="ExternalInput").ap()
    out_d = nc.dram_tensor("out", [sq, d], bf16, kind="ExternalOutput").ap()

    with tile.TileContext(nc) as tc, ExitStack() as ctx:
        const = ctx.enter_context(tc.tile_pool(name="const", bufs=1))
        ptp = ctx.enter_context(tc.tile_pool(name="ptp", bufs=7))
        outp = ctx.enter_context(tc.tile_pool(name="outp", bufs=4))
        bcp = ctx.enter_context(tc.tile_pool(name="bcp", bufs=2))
        dnp = ctx.enter_context(tc.tile_pool(name="dnp", bufs=2))
        # PSUM: score/o-proj tiles (2x2 banks), proj/denominator tiles
        # (2x1), ctx accumulators (2x1) -> exactly 8 banks.  Projections
        # get their own pool so their matmuls never wait on the exp chain
        # that consumes score tiles.
        sblk = ctx.enter_context(tc.tile_pool(name="sblk", bufs=2,
                                              space="PSUM"))
        prj = ctx.enter_context(tc.tile_pool(name="prj", bufs=2,
                                             space="PSUM"))
        ctxq = ctx.enter_context(tc.tile_pool(name="ctxq", bufs=2,
                                              space="PSUM"))

        # ---------------- DMAs first: weights, then the x tensors streamed
        # in 512-token batches round-robin (K, Q, V) so each attention
        # chunk's dependencies land just in time and nothing downstream
        # ever waits on a full-tensor load
        nb = sq // QCH
        xk_t = [const.tile([128, kc_n, QCH], bf16, tag=f"xk{b}",
                           name=f"xk{b}") for b in range(nb)]
        xq_t = [const.tile([128, kc_n, QCH], bf16, tag=f"xq{b}",
                           name=f"xq{b}") for b in range(nb)]
        xv_t = [const.tile([128, kc_n, QCH], bf16, tag=f"xv{b}",
                           name=f"xv{b}") for b in range(nb)]
        xk_r = xkT.rearrange("(c p) t -> p c t", p=128)
        xq_r = xqT.rearrange("(c p) t -> p c t", p=128)
        xv_r = xvT.rearrange("(c p) t -> p c t", p=128)

        def tsl(b):
            return slice(b * QCH, (b + 1) * QCH)

        # critical path first: K0 and Q0 unblock the scalar exp chain
        wk_sb = const.tile([128, kc_n, dkc], bf16, tag="wk")
        nc.sync.dma_start(wk_sb[:], wk_d.rearrange("(c p) m -> p c m", p=128))
        nc.sync.dma_start(xk_t[0][:], xk_r[:, :, tsl(0)])
        wq_sb = const.tile([128, kc_n, dkc], bf16, tag="wq")
        nc.sync.dma_start(wq_sb[:], wq_d.rearrange("(c p) m -> p c m", p=128))
        nc.sync.dma_start(xq_t[0][:], xq_r[:, :, tsl(0)])
        wv_sb = const.tile([128, kc_n, dkc], bf16, tag="wv")
        nc.sync.dma_start(wv_sb[:], wv_d.rearrange("(c p) m -> p c m", p=128))
        mask_i = const.tile([128, kt_n], i32, tag="mask_i")
        nc.sync.dma_start(mask_i[:], mask_d.rearrange("(t p) -> p t", p=128))
        nc.sync.dma_start(xv_t[0][:], xv_r[:, :, tsl(0)])
        for b in range(1, nb):
            nc.sync.dma_start(xk_t[b][:], xk_r[:, :, tsl(b)])
            nc.sync.dma_start(xq_t[b][:], xq_r[:, :, tsl(b)])
            nc.sync.dma_start(xv_t[b][:], xv_r[:, :, tsl(b)])
        wo_sb = const.tile([128, mc_n, fc_n, 512], f32r, tag="wo")
        nc.sync.dma_start(wo_sb[:], wo_d.rearrange("(c p) (f n) -> p c f n",
                                                   p=128, n=512))

        # ---------------- constants / persistent tensors
        ones_f = const.tile([1, 64], f32, tag="ones_f")
        nc.vector.memset(ones_f[:], 1.0)
        ones_sb = const.tile([1, 64], f32r, tag="ones")
        nc.vector.tensor_copy(ones_sb[:], ones_f[:])
        # parity masks: select one 64-partition half, zero the other
        pmask = [const.tile([128, 1], f32, tag=f"pm{i}", name=f"pm{i}")
                 for i in range(2)]
        for i in range(2):
            nc.vector.memset(pmask[i][:], 1.0)
            nc.vector.memset(pmask[i][64 * (1 - i):64 * (2 - i), :], 0.0)
        expb = const.tile([128, 1], f32, tag="expb")
        nc.vector.memset(expb[:], EXP_BIAS)
        qT_sb = const.tile([128, hpc, sq], bf16, tag="qT")
        # kT keeps head PAIRS stacked (no zero-masking needed: the qT side
        # of each score matmul has zeros in the other 64-partition half,
        # so the contraction ignores the foreign head's rows)
        kT_sb = const.tile([128, mc_n, sk], bf16, tag="kT")
        v_sb = const.tile([128, kt_n, hpc, 128], fp8, tag="v")
        nc.gpsimd.memset(v_sb[:, :, :, vw:], 0.0)
        ktb_n = min(kt_n, ktb)
        v_bf = const.tile([128, ktb_n, hpc, vw], bf16, tag="vbf")
        cxa = [const.tile([128, sq], f32r, tag=f"cx{m}", name=f"cx{m}")
               for m in range(mc_n)]
        mask01 = const.tile([128, kt_n], f32, tag="mask01")
        nc.vector.tensor_copy(mask01[:], mask_i[:])

        # ---------------- projection emitters
        def proj_qc(x_t, w_sb, dst, qc, pair):
            """One 512-token chunk of the K or Q projection (q-major).
            pair=True evicts the head pair as one unmasked copy (K)."""
            for m in range(mc_n):
                pk = prj.tile([128, 512], f32, tag="prj", name="pk")
                for c in range(kc_n):
                    nc.tensor.matmul(
                        pk[:], w_sb[:, c, m * 128:(m + 1) * 128],
                        x_t[qc][:, c, :],
                        start=(c == 0), stop=(c == kc_n - 1))
                qs = slice(qc * 512, (qc + 1) * 512)
                if pair:
                    nc.vector.tensor_copy(dst[:, m, qs], pk[:])
                else:
                    nc.scalar.mul(dst[:, 2 * m, qs], pk[:], pmask[0][:])
                    nc.vector.tensor_scalar(
                        out=dst[:, 2 * m + 1, qs], in0=pk[:],
                        scalar1=pmask[1][:], scalar2=None, op0=mult)

        def vproj_tile(t):
            """V projection for one k-tile."""
            pv = prj.tile([128, dkc], f32, tag="prj", name="pv")
            tr = slice((t % ktb) * 128, (t % ktb + 1) * 128)
            for c in range(kc_n):
                nc.tensor.matmul(pv[:], xv_t[t // ktb][:, c, tr],
                                 wv_sb[:, c, :],
                                 start=(c == 0), stop=(c == kc_n - 1))
            nc.vector.tensor_scalar(
                out=v_sb[:, t, :, 0:DK],
                in0=pv[:].rearrange("p (h k) -> p h k", h=hpc),
                scalar1=mask01[:, t:t + 1], scalar2=None, op0=mult)
            nc.vector.tensor_copy(
                v_sb[:, t, :, DK:vw],
                mask01[:, t:t + 1].unsqueeze(1)
                .broadcast_to([128, hpc, 1]))
            if t < ktb_n:
                nc.vector.tensor_scalar(
                    out=v_bf[:, t, :, 0:DK],
                    in0=pv[:].rearrange("p (h k) -> p h k", h=hpc),
                    scalar1=mask01[:, t:t + 1], scalar2=None, op0=mult)
                nc.vector.tensor_copy(
                    v_bf[:, t, :, DK:vw],
                    mask01[:, t:t + 1].unsqueeze(1)
                    .broadcast_to([128, hpc, 1]))

        def vproj_batch(b):
            """V projection for k-tiles [b*ktb, (b+1)*ktb)."""
            for t in range(b * ktb, min((b + 1) * ktb, kt_n)):
                vproj_tile(t)

        # ---------------- attention, q-chunk major
        # Per q-chunk the (head, block) units are flattened into one list
        # and the AV matmul of unit i is emitted after the score matmuls
        # of unit i+2 (and normalization of head j inside head j+1), so the
        # exp -> causal-select chain hides under later score matmuls.
        def attention_qc(qc, fillers=()):
            q0 = qc * QCH
            nkt = (q0 + QCH) // 128           # ktiles needed (causal bound)
            nblk = nkt // 2
            use_fp8 = qc > 0                   # chunk 0 holds the short rows
            fillers = list(fillers)
            deferred = []

            def mk_av(cx_ps, pB, j, blk):
                def go():
                    if use_fp8:
                        nc.tensor.matmul(cx_ps[:],
                                         v_sb[:, 2 * blk:2 * blk + 2, j, :],
                                         pB[:], perf_mode=DR,
                                         start=(blk == 0),
                                         stop=(blk == nblk - 1))
                    else:
                        for t2 in range(2):
                            kt = blk * 2 + t2
                            nc.tensor.matmul(cx_ps[:], v_bf[:, kt, j, :],
                                             pB[:, t2, :],
                                             start=(kt == 0),
                                             stop=(kt == nkt - 1))
                return go

            def mk_norm(cx_ps, j):
                def go():
                    pb = (j % 2) * 64
                    ms = j // 2
                    dn = dnp.tile([1, QCH], f32r, tag="dn", name="dn")
                    nc.vector.tensor_scalar_add(dn[:], cx_ps[DK:DK + 1, :],
                                                DEN_EPS)
                    bc_ps = prj.tile([64, QCH], f32, tag="prj", name="bc_ps")
                    nc.tensor.matmul(bc_ps[:], ones_sb[:], dn[:],
                                     start=True, stop=True)
                    bc = bcp.tile([64, QCH], f32, tag="bc", name="bc")
                    nc.vector.reciprocal_approx_fast(bc[:], bc_ps[:])
                    nc.vector.tensor_tensor(
                        out=cxa[ms][pb:pb + 64, q0:q0 + QCH],
                        in0=cx_ps[0:DK, :], in1=bc[:], op=mult)
                return go

            for j in range(hpc):
                cx_np = 128 if use_fp8 else vw
                cx_ps = ctxq.tile([cx_np, QCH], f32, tag="ctx", name="cx_ps")
                def score_mm(t2, kt, qlo):
                    nc.tensor.matmul(
                        sB[:, t2, qlo:],
                        kT_sb[:, j // 2, kt * 128:(kt + 1) * 128],
                        qT_sb[:, j, q0 + qlo:q0 + QCH],
                        start=True, stop=True)

                for blk in range(nblk):
                    sB = sblk.tile([128, 2, 512], f32, tag="s", name="sB")
                    pB = ptp.tile([128, 2, 512], fp8 if use_fp8 else bf16,
                                  tag="p", name="pB")
                    if blk == nblk - 1:
                        # last (diagonal) block: q-quarter [q0, q0+256) is
                        # entirely future -> skip its scores and exp
                        for t2 in range(2):
                            score_mm(t2, blk * 2 + t2, 256)
                        nc.gpsimd.memset(pB[:, :, 0:256], 0.0)
                        nc.scalar.activation(pB[:, :, 256:], sB[:, :, 256:],
                                             Exp, scale=0.125, bias=expb[:])
                        nc.gpsimd.affine_select(
                            out=pB[:, :, 256:], in_=pB[:, :, 256:],
                            compare_op=is_ge, fill=0.0,
                            base=q0 + 256 - blk * 256, channel_multiplier=-1,
                            pattern=[[-128, 2], [1, 256]])
                    else:
                        for t2 in range(2):
                            score_mm(t2, blk * 2 + t2, 0)
                        nc.scalar.activation(pB[:], sB[:], Exp,
                                             scale=0.125, bias=expb[:])
                        if blk == nblk - 2:
                            # q-half [q0+256, q0+512) is entirely past this
                            # block's keys -> only the first half needs the
                            # causal select (shortens the exp->AV chain)
                            nc.gpsimd.affine_select(
                                out=pB[:, :, 0:256], in_=pB[:, :, 0:256],
                                compare_op=is_ge,
                                fill=0.0, base=q0 - blk * 256,
                                channel_multiplier=-1,
                                pattern=[[-128, 2], [1, 256]])
                    deferred.append(mk_av(cx_ps, pB, j, blk))
                    while len(deferred) > 3:
                        deferred.pop(0)()
                deferred.append(mk_norm(cx_ps, j))
                # next-chunk projection work rides in the exp-paced gaps;
                # emitting it here (not at the chunk boundary) keeps the
                # scalar exp chain from starving behind it in the PE queue
                npop = -(-len(fillers) // (hpc - j))
                for _ in range(npop):
                    fillers.pop(0)()
            for fn in deferred:
                fn()
            for fn in fillers:
                fn()

        def oproj_qt(qc, qt, on_scalar=False):
            qg = qc * QCH + qt * 128
            po = sblk.tile([128, fc_n, 512], f32, tag="s", name="po")
            for fc in range(fc_n):
                for m in range(mc_n):
                    nc.tensor.matmul(
                        po[:, fc, :], cxa[m][:, qg:qg + 128],
                        wo_sb[:, m, fc, :],
                        start=(m == 0), stop=(m == mc_n - 1))
            o_sb = outp.tile([128, fc_n, 512], bf16, tag="o", name="o_sb")
            if on_scalar:
                nc.scalar.copy(o_sb[:], po[:])
            else:
                nc.vector.tensor_copy(o_sb[:], po[:])
            nc.sync.dma_start(out_d[qg:qg + 128, :],
                              o_sb[:].rearrange("p f n -> p (f n)"))

        def oproj_qc(qc, on_scalar=False):
            for qt in range(QCH // 128):
                oproj_qt(qc, qt, on_scalar)

        # interleaved schedule: chunk qc's attention carries chunk qc+1's
        # K/Q/V projections AND chunk qc-1's output projection as per-head
        # fillers, so no multi-microsecond PE lump ever sits between one
        # chunk's exps and the next chunk's scores
        def mk_proj(x_t, w_sb, dst, qc, pair):
            return lambda: proj_qc(x_t, w_sb, dst, qc, pair)

        def mk_vhalf(b, half):
            def go():
                lo = b * ktb + half * (ktb // 2)
                for t in range(lo, min(lo + max(ktb // 2, 1), kt_n)):
                    vproj_tile(t)
            return go

        def mk_ohalf(qc, half):
            def go():
                for qt in range(2 * half, min(2 * half + 2, QCH // 128)):
                    oproj_qt(qc, qt)
            return go

        proj_qc(xk_t, wk_sb, kT_sb, 0, True)
        proj_qc(xq_t, wq_sb, qT_sb, 0, False)
        vproj_batch(0)
        for qc in range(qc_n):
            fillers = []
            if qc + 1 < qc_n:
                fillers += [mk_proj(xk_t, wk_sb, kT_sb, qc + 1, True),
                            mk_proj(xq_t, wq_sb, qT_sb, qc + 1, False),
                            mk_vhalf(qc + 1, 0), mk_vhalf(qc + 1, 1)]
            if qc > 0:
                fillers += [mk_ohalf(qc - 1, 0), mk_ohalf(qc - 1, 1)]
            attention_qc(qc, fillers)
        for b in range(qc_n, (kt_n + ktb - 1) // ktb):
            vproj_batch(b)            # small-cfg safety: leftover v tiles
        oproj_qc(qc_n - 1, on_scalar=True)
    nc.compile()
    return nc


def _get_program(cfg):
    if cfg not in _PROG_CACHE:
        _PROG_CACHE[cfg] = _build(cfg)
    return _PROG_CACHE[cfg]


def _shard_inputs(query, key, value, mask, Wq, Wk, Wv, Wo):
    """Build the 8 per-core input maps (bf16 on the host)."""
    import ml_dtypes
    bf = ml_dtypes.bfloat16
    in_maps = []
    xt = {}
    for b in range(B):
        xt[b] = (np.ascontiguousarray(query[b].T).astype(bf),
                 np.ascontiguousarray(key[b].T).astype(bf),
                 np.ascontiguousarray(value[b].T).astype(bf),
                 np.ascontiguousarray(mask[b], dtype=np.int32))
    for c in range(N_CORES):
        b, hg = divmod(c, CORES_PER_BATCH)
        rows = slice(hg * DKC, (hg + 1) * DKC)
        xq, xk, xv, mb = xt[b]
        in_maps.append({
            "xqT": xq, "xkT": xk, "xvT": xv, "maskb": mb,
            "wq": np.ascontiguousarray(Wq[rows, :].T).astype(bf),
            "wk": np.ascontiguousarray(Wk[rows, :].T).astype(bf),
            "wv": np.ascontiguousarray(Wv[rows, :].T).astype(bf),
            "wo": np.ascontiguousarray(Wo[:, rows].T, dtype=np.float32),
        })
    return in_maps


def kernel(query, key, value, mask, Wq, Wk, Wv, Wo):
    from concourse.bass_utils import run_bass_kernel_spmd

    nc = _get_program((SQ, SK, D, DKC))
    in_maps = _shard_inputs(np.asarray(query), np.asarray(key),
                            np.asarray(value), np.asarray(mask),
                            np.asarray(Wq), np.asarray(Wk),
                            np.asarray(Wv), np.asarray(Wo))
    res = run_bass_kernel_spmd(nc, in_maps, list(range(N_CORES)))
    out = np.zeros((B, SQ, D), dtype=np.float32)
    for c in range(N_CORES):
        out[c // CORES_PER_BATCH] += np.asarray(
            res.results[c]["out"]).astype(np.float32)
    return out


# revision 101
# speedup vs baseline: 1.1767x; 1.1767x over previous
"""Multi-head attention (B=2, SQ=SK=2048, D=1024, H=16, DK=64) on 8 TRN2 cores.

Sharding: core c handles batch b = c//4 and head-group hg = c%4 (4 heads,
256 feature columns of each projection).  Each core computes its heads'
Q/K/V projections, causal+padding-masked softmax attention, and a partial
output projection; the host sums the 4 partials per batch.

Schedule: the kernel is emitted q-chunk-major with K/Q projections,
V projection k-tile batches, attention, and the (one chunk deferred)
output projection interleaved:
    K0 Q0 V0 A0 | K1 Q1 V1 A1 O0 | ... | K3 Q3 V3 A3 O2 | O3
so the scalar-engine exp chain (the attention pacer) starts as soon as
the first 512 tokens of K and Q are projected, and all PE work after
that point fills the gaps the exp chain leaves.  All x tiles stay
resident in SBUF (bf16) so no DMA ever waits on compute.

Precision/rate strategy (all inputs converted on the host):
  projections + scores            bf16 operands, f32 PSUM
  AV (attn @ V), chunks >= 1      fp8 e4m3 via DoubleRow perf mode
                                  (2 k-tiles per matmul at 0.5 cyc/row)
  AV chunk 0                      bf16 (short causal rows would expose
                                  fp8 quantization noise un-averaged)
  p = exp(s/8 - 4.75)             bias cancels in the softmax ratio and
                                  keeps p_max well under the HW e4m3
                                  convert's inf threshold (240, not the
                                  448 e4m3fn max); max s/8 here is 9.29
  ctx, Wo, O-proj                 f32/f32r (full PE rate at N>=256)
  out                             bf16 partials, host sums in f32

Device layouts (per core):
  qT     [128, head, tok]  dk=64 in one 64-partition half of a 128 slot,
                           other half zeroed so score matmuls contract a
                           full K=128 (K<128 matmuls don't register for
                           the PE clock gate and run at half clock)
  kT     [128, pair, tok]  head PAIRS stacked un-masked: the qT side's
                           zero half already excludes the foreign head
                           from the contraction, halving K evictions
  v      [ktok, kt, head, 128]  fp8, rows padded to 128 (ISA rejects
                           65-wide DoubleRow LdWeights); mask folded in;
                           col 64 = masked ones -> softmax denominator
  sT     [ktok, qtok]      transposed scores (PSUM f32)
  pT     exp(sT/8-4.75)    causal: affine_select on diagonal blocks; the
                           fully-masked q-quarter of the last block is
                           memset instead of exp'd (scalar is the pacer)
  ctxT   [dk+1, qtok]      PSUM f32 accumulated over k-tile (pairs)
  out    [qtok, D]         bf16 partial, host sums the 4 head-groups
"""

import numpy as np

B, SQ, SK, D, H, DK = 2, 2048, 2048, 1024, 16, 64
N_CORES = 8
CORES_PER_BATCH = 4
DKC = D // CORES_PER_BATCH          # 256 projection columns per core
QCH = 512                           # q-chunk (moving free dim)
DEN_EPS = 1e-9
EXP_BIAS = -4.75

_PROG_CACHE = {}


def _build(cfg):
    """Build the per-core Bass program. cfg = (sq, sk, d, dkc)."""
    import concourse.bass as bass  # noqa: F401
    import concourse.mybir as mybir
    import concourse.tile as tile
    from concourse import bacc
    from contextlib import ExitStack

    f32 = mybir.dt.float32
    f32r = mybir.dt.float32r
    bf16 = mybir.dt.bfloat16
    fp8 = mybir.dt.float8e4
    i32 = mybir.dt.int32
    Exp = mybir.ActivationFunctionType.Exp
    mult = mybir.AluOpType.mult
    is_ge = mybir.AluOpType.is_ge
    DR = mybir.MatmulPerfMode.DoubleRow

    sq, sk, d, dkc = cfg
    kc_n = d // 128                  # contraction chunks for projections
    mc_n = dkc // 128                # 128-wide dk chunks (q/k layout)
    kt_n = sk // 128                 # key tiles
    qc_n = sq // QCH                 # q chunks
    hpc = dkc // DK                  # heads per core
    vw = DK + 1                      # v row width per head incl. ones col
    fc_n = d // 512                  # output feature chunks
    ktb = QCH // 128                 # v k-tiles per interleave batch

    nc = bacc.Bacc("TRN2", target_bir_lowering=False, debug=False,
                   enable_asserts=False, num_devices=N_CORES)

    xqT = nc.dram_tensor("xqT", [d, sq], bf16, kind="ExternalInput").ap()
    xkT = nc.dram_tensor("xkT", [d, sk], bf16, kind="ExternalInput").ap()
    xvT = nc.dram_tensor("xvT", [d, sk], bf16, kind="ExternalInput").ap()
    wq_d = nc.dram_tensor("wq", [d, dkc], bf16, kind="ExternalInput").ap()
    wk_d = nc.dram_tensor("wk", [d, dkc], bf16, kind="ExternalInput").ap()
    wv_d = nc.dram_tensor("wv", [d, dkc], bf16, kind="ExternalInput").ap()
    wo_d = nc.dram_tensor("wo", [dkc, d], f32r, kind="ExternalInput").ap()
    mask_d = nc.dram_tensor("maskb", [sk], i32, kind="ExternalInput").ap()
    out_d = nc.dram_tensor("out", [sq, d], bf16, kind="ExternalOutput").ap()

    with tile.TileContext(nc) as tc, ExitStack() as ctx:
        const = ctx.enter_context(tc.tile_pool(name="const", bufs=1))
        ptp = ctx.enter_context(tc.tile_pool(name="ptp", bufs=6))
        outp = ctx.enter_context(tc.tile_pool(name="outp", bufs=4))
        bcp = ctx.enter_context(tc.tile_pool(name="bcp", bufs=2))
        dnp = ctx.enter_context(tc.tile_pool(name="dnp", bufs=2))
        # PSUM: score/o-proj tiles (2x2 banks), proj/denominator tiles
        # (2x1), ctx accumulators (2x1) -> exactly 8 banks.  Projections
        # get their own pool so their matmuls never wait on the exp chain
        # that consumes score tiles.
        sblk = ctx.enter_context(tc.tile_pool(name="sblk", bufs=2,
                                              space="PSUM"))
        prj = ctx.enter_context(tc.tile_pool(name="prj", bufs=2,
                                             space="PSUM"))
        ctxq = ctx.enter_context(tc.tile_pool(name="ctxq", bufs=2,
                                              space="PSUM"))

        # ---------------- DMAs first: weights, then the x tensors streamed
        # in 512-token batches round-robin (K, Q, V) so each attention
        # chunk's dependencies land just in time and nothing downstream
        # ever waits on a full-tensor load
        nb = sq // QCH
        xk_t = [const.tile([128, kc_n, QCH], bf16, tag=f"xk{b}",
                           name=f"xk{b}") for b in range(nb)]
        xq_t = [const.tile([128, kc_n, QCH], bf16, tag=f"xq{b}",
                           name=f"xq{b}") for b in range(nb)]
        xv_t = [const.tile([128, kc_n, QCH], bf16, tag=f"xv{b}",
                           name=f"xv{b}") for b in range(nb)]
        xk_r = xkT.rearrange("(c p) t -> p c t", p=128)
        xq_r = xqT.rearrange("(c p) t -> p c t", p=128)
        xv_r = xvT.rearrange("(c p) t -> p c t", p=128)

        def tsl(b):
            return slice(b * QCH, (b + 1) * QCH)

        # critical path first: K0 and Q0 unblock the scalar exp chain
        wk_sb = const.tile([128, kc_n, dkc], bf16, tag="wk")
        nc.sync.dma_start(wk_sb[:], wk_d.rearrange("(c p) m -> p c m", p=128))
        nc.sync.dma_start(xk_t[0][:], xk_r[:, :, tsl(0)])
        wq_sb = const.tile([128, kc_n, dkc], bf16, tag="wq")
        nc.sync.dma_start(wq_sb[:], wq_d.rearrange("(c p) m -> p c m", p=128))
        nc.sync.dma_start(xq_t[0][:], xq_r[:, :, tsl(0)])
        wv_sb = const.tile([128, kc_n, dkc], bf16, tag="wv")
        nc.sync.dma_start(wv_sb[:], wv_d.rearrange("(c p) m -> p c m", p=128))
        mask_i = const.tile([128, kt_n], i32, tag="mask_i")
        nc.sync.dma_start(mask_i[:], mask_d.rearrange("(t p) -> p t", p=128))
        nc.sync.dma_start(xv_t[0][:], xv_r[:, :, tsl(0)])
        for b in range(1, nb):
            nc.sync.dma_start(xk_t[b][:], xk_r[:, :, tsl(b)])
            nc.sync.dma_start(xq_t[b][:], xq_r[:, :, tsl(b)])
            nc.sync.dma_start(xv_t[b][:], xv_r[:, :, tsl(b)])
        wo_sb = const.tile([128, mc_n, fc_n, 512], f32r, tag="wo")
        nc.sync.dma_start(wo_sb[:], wo_d.rearrange("(c p) (f n) -> p c f n",
                                                   p=128, n=512))

        # ---------------- constants / persistent tensors
        ones_f = const.tile([1, 64], f32, tag="ones_f")
        nc.vector.memset(ones_f[:], 1.0)
        ones_sb = const.tile([1, 64], f32r, tag="ones")
        nc.vector.tensor_copy(ones_sb[:], ones_f[:])
        # parity masks: select one 64-partition half, zero the other
        pmask = [const.tile([128, 1], f32, tag=f"pm{i}", name=f"pm{i}")
                 for i in range(2)]
        for i in range(2):
            nc.vector.memset(pmask[i][:], 1.0)
            nc.vector.memset(pmask[i][64 * (1 - i):64 * (2 - i), :], 0.0)
        expb = const.tile([128, 1], f32, tag="expb")
        nc.vector.memset(expb[:], EXP_BIAS)
        qT_sb = const.tile([128, hpc, sq], bf16, tag="qT")
        # kT keeps head PAIRS stacked (no zero-masking needed: the qT side
        # of each score matmul has zeros in the other 64-partition half,
        # so the contraction ignores the foreign head's rows)
        kT_sb = const.tile([128, mc_n, sk], bf16, tag="kT")
        v_sb = const.tile([128, kt_n, hpc, 128], fp8, tag="v")
        nc.gpsimd.memset(v_sb[:, :, :, vw:], 0.0)
        ktb_n = min(kt_n, ktb)
        v_bf = const.tile([128, ktb_n, hpc, vw], bf16, tag="vbf")
        cxa = [const.tile([128, sq], f32r, tag=f"cx{m}", name=f"cx{m}")
               for m in range(mc_n)]
        mask01 = const.tile([128, kt_n], f32, tag="mask01")
        nc.vector.tensor_copy(mask01[:], mask_i[:])

        # ---------------- projection emitters
        def proj_qc(x_t, w_sb, dst, qc, pair):
            """One 512-token chunk of the K or Q projection (q-major).
            pair=True evicts the head pair as one unmasked copy (K)."""
            for m in range(mc_n):
                pk = prj.tile([128, 512], f32, tag="prj", name="pk")
                for c in range(kc_n):
                    nc.tensor.matmul(
                        pk[:], w_sb[:, c, m * 128:(m + 1) * 128],
                        x_t[qc][:, c, :],
                        start=(c == 0), stop=(c == kc_n - 1))
                qs = slice(qc * 512, (qc + 1) * 512)
                if pair:
                    nc.vector.tensor_copy(dst[:, m, qs], pk[:])
                else:
                    nc.scalar.mul(dst[:, 2 * m, qs], pk[:], pmask[0][:])
                    nc.vector.tensor_scalar(
                        out=dst[:, 2 * m + 1, qs], in0=pk[:],
                        scalar1=pmask[1][:], scalar2=None, op0=mult)

        def vproj_tile(t):
            """V projection for one k-tile."""
            pv = prj.tile([128, dkc], f32, tag="prj", name="pv")
            tr = slice((t % ktb) * 128, (t % ktb + 1) * 128)
            for c in range(kc_n):
                nc.tensor.matmul(pv[:], xv_t[t // ktb][:, c, tr],
                                 wv_sb[:, c, :],
                                 start=(c == 0), stop=(c == kc_n - 1))
            nc.vector.tensor_scalar(
                out=v_sb[:, t, :, 0:DK],
                in0=pv[:].rearrange("p (h k) -> p h k", h=hpc),
                scalar1=mask01[:, t:t + 1], scalar2=None, op0=mult)
            nc.vector.tensor_copy(
                v_sb[:, t, :, DK:vw],
                mask01[:, t:t + 1].unsqueeze(1)
                .broadcast_to([128, hpc, 1]))
            if t < ktb_n:
                nc.vector.tensor_scalar(
                    out=v_bf[:, t, :, 0:DK],
                    in0=pv[:].rearrange("p (h k) -> p h k", h=hpc),
                    scalar1=mask01[:, t:t + 1], scalar2=None, op0=mult)
                nc.vector.tensor_copy(
                    v_bf[:, t, :, DK:vw],
                    mask01[:, t:t + 1].unsqueeze(1)
                    .broadcast_to([128, hpc, 1]))

        def vproj_batch(b):
            """V projection for k-tiles [b*ktb, (b+1)*ktb)."""
            for t in range(b * ktb, min((b + 1) * ktb, kt_n)):
                vproj_tile(t)

        # ---------------- attention, q-chunk major
        # Per q-chunk the (head, block) units are flattened into one list
        # and the AV matmul of unit i is emitted after the score matmuls
        # of unit i+2 (and normalization of head j inside head j+1), so the
        # exp -> causal-select chain hides under later score matmuls.
        def attention_qc(qc, fillers=()):
            q0 = qc * QCH
            nkt = (q0 + QCH) // 128           # ktiles needed (causal bound)
            nblk = nkt // 2
            use_fp8 = qc > 0                   # chunk 0 holds the short rows
            fillers = list(fillers)
            deferred = []

            def mk_av(cx_ps, pB, j, blk):
                def go():
                    if use_fp8:
                        nc.tensor.matmul(cx_ps[:],
                                         v_sb[:, 2 * blk:2 * blk + 2, j, :],
                                         pB[:], perf_mode=DR,
                                         start=(blk == 0),
                                         stop=(blk == nblk - 1))
                    else:
                        for t2 in range(2):
                            kt = blk * 2 + t2
                            nc.tensor.matmul(cx_ps[:], v_bf[:, kt, j, :],
                                             pB[:, t2, :],
                                             start=(kt == 0),
                                             stop=(kt == nkt - 1))
                return go

            def mk_norm(cx_ps, j):
                def go():
                    pb = (j % 2) * 64
                    ms = j // 2
                    dn = dnp.tile([1, QCH], f32r, tag="dn", name="dn")
                    nc.vector.tensor_scalar_add(dn[:], cx_ps[DK:DK + 1, :],
                                                DEN_EPS)
                    bc_ps = prj.tile([64, QCH], f32, tag="prj", name="bc_ps")
                    nc.tensor.matmul(bc_ps[:], ones_sb[:], dn[:],
                                     start=True, stop=True)
                    bc = bcp.tile([64, QCH], f32, tag="bc", name="bc")
                    nc.vector.reciprocal_approx_fast(bc[:], bc_ps[:])
                    nc.vector.tensor_tensor(
                        out=cxa[ms][pb:pb + 64, q0:q0 + QCH],
                        in0=cx_ps[0:DK, :], in1=bc[:], op=mult)
                return go

            for j in range(hpc):
                cx_np = 128 if use_fp8 else vw
                cx_ps = ctxq.tile([cx_np, QCH], f32, tag="ctx", name="cx_ps")
                def score_mm(t2, kt, qlo):
                    nc.tensor.matmul(
                        sB[:, t2, qlo:],
                        kT_sb[:, j // 2, kt * 128:(kt + 1) * 128],
                        qT_sb[:, j, q0 + qlo:q0 + QCH],
                        start=True, stop=True)

                for blk in range(nblk):
                    sB = sblk.tile([128, 2, 512], f32, tag="s", name="sB")
                    pB = ptp.tile([128, 2, 512], fp8 if use_fp8 else bf16,
                                  tag="p", name="pB")
                    if blk == nblk - 1:
                        # last (diagonal) block: q-quarter [q0, q0+256) is
                        # entirely future -> skip its scores and exp
                        for t2 in range(2):
                            score_mm(t2, blk * 2 + t2, 256)
                        nc.gpsimd.memset(pB[:, :, 0:256], 0.0)
                        nc.scalar.activation(pB[:, :, 256:], sB[:, :, 256:],
                                             Exp, scale=0.125, bias=expb[:])
                        nc.gpsimd.affine_select(
                            out=pB[:, :, 256:], in_=pB[:, :, 256:],
                            compare_op=is_ge, fill=0.0,
                            base=q0 + 256 - blk * 256, channel_multiplier=-1,
                            pattern=[[-128, 2], [1, 256]])
                    else:
                        for t2 in range(2):
                            score_mm(t2, blk * 2 + t2, 0)
                        nc.scalar.activation(pB[:], sB[:], Exp,
                                             scale=0.125, bias=expb[:])
                        if blk == nblk - 2:
                            # q-half [q0+256, q0+512) is entirely past this
                            # block's keys -> only the first half needs the
                            # causal select (shortens the exp->AV chain)
                            nc.gpsimd.affine_select(
                                out=pB[:, :, 0:256], in_=pB[:, :, 0:256],
                                compare_op=is_ge,
                                fill=0.0, base=q0 - blk * 256,
                                channel_multiplier=-1,
                                pattern=[[-128, 2], [1, 256]])
                    deferred.append(mk_av(cx_ps, pB, j, blk))
                    while len(deferred) > 3:
                        deferred.pop(0)()
                deferred.append(mk_norm(cx_ps, j))
                # next-chunk projection work rides in the exp-paced gaps;
                # emitting it here (not at the chunk boundary) keeps the
                # scalar exp chain from starving behind it in the PE queue
                npop = -(-len(fillers) // (hpc - j))
                for _ in range(npop):
                    fillers.pop(0)()
            for fn in deferred:
                fn()
            for fn in fillers:
                fn()

        def oproj_qt(qc, qt, on_scalar=False):
            qg = qc * QCH + qt * 128
            po = sblk.tile([128, fc_n, 512], f32, tag="s", name="po")
            for fc in range(fc_n):
                for m in range(mc_n):
                    nc.tensor.matmul(
                        po[:, fc, :], cxa[m][:, qg:qg + 128],
                        wo_sb[:, m, fc, :],
                        start=(m == 0), stop=(m == mc_n - 1))
            o_sb = outp.tile([128, fc_n, 512], bf16, tag="o", name="o_sb")
            if on_scalar:
                nc.scalar.copy(o_sb[:], po[:])
            else:
                nc.vector.tensor_copy(o_sb[:], po[:])
            nc.sync.dma_start(out_d[qg:qg + 128, :],
                              o_sb[:].rearrange("p f n -> p (f n)"))

        def oproj_qc(qc, on_scalar=False):
            for qt in range(QCH // 128):
                oproj_qt(qc, qt, on_scalar)

        # interleaved schedule: chunk qc's attention carries chunk qc+1's
        # K/Q/V projections AND chunk qc-1's output projection as per-head
        # fillers, so no multi-microsecond PE lump ever sits between one
        # chunk's exps and the next chunk's scores
        def mk_proj(x_t, w_sb, dst, qc, pair):
            return lambda: proj_qc(x_t, w_sb, dst, qc, pair)

        def mk_vhalf(b, half):
            def go():
                lo = b * ktb + half * (ktb // 2)
                for t in range(lo, min(lo + max(ktb // 2, 1), kt_n)):
                    vproj_tile(t)
            return go

        def mk_ohalf(qc, half):
            def go():
                for qt in range(2 * half, min(2 * half + 2, QCH // 128)):
                    oproj_qt(qc, qt)
            return go

        proj_qc(xk_t, wk_sb, kT_sb, 0, True)
        proj_qc(xq_t, wq_sb, qT_sb, 0, False)
        vproj_batch(0)
        for qc in range(qc_n):
            fillers = []
            if qc + 1 < qc_n:
                fillers += [mk_proj(xk_t, wk_sb, kT_sb, qc + 1, True),
                            mk_proj(xq_t, wq_sb, qT_sb, qc + 1, False),
                            mk_vhalf(qc + 1, 0), mk_vhalf(qc + 1, 1)]
            if qc > 0:
                fillers += [mk_ohalf(qc - 1, 0), mk_ohalf(qc - 1, 1)]
            attention_qc(qc, fillers)
        for b in range(qc_n, (kt_n + ktb - 1) // ktb):
            vproj_batch(b)            # small-cfg safety: leftover v tiles
        oproj_qc(qc_n - 1, on_scalar=True)
    nc.compile()
    return nc


def _get_program(cfg):
    if cfg not in _PROG_CACHE:
        _PROG_CACHE[cfg] = _build(cfg)
    return _PROG_CACHE[cfg]


def _shard_inputs(query, key, value, mask, Wq, Wk, Wv, Wo):
    """Build the 8 per-core input maps (bf16 on the host)."""
    import ml_dtypes
    bf = ml_dtypes.bfloat16
    in_maps = []
    xt = {}
    for b in range(B):
        xt[b] = (np.ascontiguousarray(query[b].T).astype(bf),
                 np.ascontiguousarray(key[b].T).astype(bf),
                 np.ascontiguousarray(value[b].T).astype(bf),
                 np.ascontiguousarray(mask[b], dtype=np.int32))
    for c in range(N_CORES):
        b, hg = divmod(c, CORES_PER_BATCH)
        rows = slice(hg * DKC, (hg + 1) * DKC)
        xq, xk, xv, mb = xt[b]
        in_maps.append({
            "xqT": xq, "xkT": xk, "xvT": xv, "maskb": mb,
            "wq": np.ascontiguousarray(Wq[rows, :].T).astype(bf),
            "wk": np.ascontiguousarray(Wk[rows, :].T).astype(bf),
            "wv": np.ascontiguousarray(Wv[rows, :].T).astype(bf),
            "wo": np.ascontiguousarray(Wo[:, rows].T, dtype=np.float32),
        })
    return in_maps


def kernel(query, key, value, mask, Wq, Wk, Wv, Wo):
    from concourse.bass_utils import run_bass_kernel_spmd

    nc = _get_program((SQ, SK, D, DKC))
    in_maps = _shard_inputs(np.asarray(query), np.asarray(key),
                            np.asarray(value), np.asarray(mask),
                            np.asarray(Wq), np.asarray(Wk),
                            np.asarray(Wv), np.asarray(Wo))
    res = run_bass_kernel_spmd(nc, in_maps, list(range(N_CORES)))
    out = np.zeros((B, SQ, D), dtype=np.float32)
    for c in range(N_CORES):
        out[c // CORES_PER_BATCH] += np.asarray(
            res.results[c]["out"]).astype(np.float32)
    return out
